# revision 27
# baseline (speedup 1.0000x reference)
"""Trainium2 Bass kernel for nn_DualAttention (S=2048, B=16, H2=2048, V=1024).

Computation (per the reference):
    sum_w = hidden @ Ww + bw + z @ Wz + bz + w_a*0.5        [S, B, V]
    u     = tanh(sum_w) @ Vw + vb                            [S, B, 1]
    out   = softmax(u, axis=0)                               [S, B, 1]

Strategy
--------
Data-parallel over batch: 16 batches -> 2 per NeuronCore (8 cores).
Host-side prep per core (fp16 operands; PE fp16 runs at bf16 rate with
11 mantissa bits -> ~1.5e-3 rel err):
  * X [ROWS=4096, H=4096] = concat(hidden, z) along hidden, rows
    b-major (row = b_local*2048 + s); pre-tiled to xtt [32, P, NK*128]
    so each 128-row block is one contiguous 1 MiB DMA
  * W [H, V] = concat([Ww, Wz], 0), tiled wt [2, NK, P, 512] (v-halves)
  * bias = bw + bz + 0.5*w_a and Vw, both replicated across the 128
    partitions ([P, V] f32) for free-axis use on the vector engine
X-stationary device kernel, psum layout [rows, v]:
  for each 128-row block rb, v-half vh:
    psum[128,512] += sum_k xtt[rb,k].T @ wt[vh,k]   (32 matmuls)
    tb = psum + bias_rep      (DVE;  ACT bias is per-partition only)
    tb = tanh(tb)             (ACT, in place)
    u_all[:, rb] = sum_v tb*vw_rep (+ prev half)  (DVE tensor_tensor_reduce)
  The tensor engine runs ONLY the 2048 main matmuls -- no M=1
  second-stage matmuls (those cost ~+100ns each in the [v, rows]
  layout). The 128-row blocks rb0..3 are processed k-chunk-major
  across 4 psum banks while W streams in, so the PE has 4x work per
  arrived DMA byte and the HAM clock-gate reaches full rate early.
Raw scores u ship per batch; exp + normalize run host-side on [S, B]
(the softmax is per batch column, so this is shard-local postprocessing;
vb is dropped: softmax is shift-invariant).
"""

import numpy as np

# ---------------------------------------------------------------------------
# Problem constants (hardcoded; kernel.py must be self-contained)
# ---------------------------------------------------------------------------
S, B, H2, V = 2048, 16, 2048, 1024
ALPHA_S = 0.5
NCORES = 8
BC = B // NCORES            # local batches per core (2)
ROWS = S * BC               # 4096 rows per core (b-major)
H = 2 * H2                  # 4096 contraction dim (hidden ++ z)
P = 128
NK = H // P                 # 32 k-tiles
NVH = 2                     # v-halves of 512
VH = V // NVH               # 512
NRB = ROWS // P             # 32 row blocks of 128
RPB = NRB // BC             # row blocks per batch (16)
WKC = 8                     # k-tiles per W DMA chunk
NWC = NK // WKC             # 4 chunks per v-half


# ---------------------------------------------------------------------------
# Workarounds for this walrus build's 1-sync-wait-per-instruction limit
# ---------------------------------------------------------------------------
def _install_drain_patch():
    import concourse.mybir as mybir
    from concourse.tile import TileContext
    from concourse.vector_clock import ScopedClock

    def _drain_and_barrier(self, tick_clock, wait_clock):
        nc = self.nc
        drain_inst = nc.sync.drain()
        wait_clock.add_sem_waits(
            drain_inst.ins, ScopedClock({None: tick_clock.global_clock})
        )
        si = drain_inst.ins.sync_info
        if si is not None:
            waits = list(si.on_wait)
            if len(waits) > 1:
                si.on_wait = [waits[0]]
                for w in waits[1:]:
                    nop = nc.sync.nop(nofuse=True)
                    nop.ins.sync_info = mybir.SyncInfo(on_wait=[w], on_update=[])
        nc.all_engine_barrier()
        assert self.sems is not None
        popped = nc._tile_sem_poison_stack.pop()
        assert popped is self._sem_poison
        nc.clear_and_free_semaphores(list(self.sems.allocated().values()))

    TileContext._drain_and_barrier = _drain_and_barrier


def _split_multiwait(nc):
    """Hoist extra sync waits onto same-engine event-semaphore instructions
    inserted just before the carrying instruction."""
    import concourse.mybir as mybir

    counter = 0
    for fn in nc.m.functions:
        for bb in fn.blocks:
            insts = bb.instructions
            new_list = []
            changed = False
            for inst in insts:
                si = inst.sync_info
                if si is not None:
                    waits = list(si.on_wait)
                    if len(waits) > 1:
                        for w in waits[:-1]:
                            counter += 1
                            nop = mybir.InstEventSemaphore(
                                name=f"I-mwsplit-{counter}"
                            )
                            nop.engine = inst.engine
                            nop.bass_nofuse = True
                            nop.sync_info = mybir.SyncInfo(
                                on_wait=[w], on_update=[]
                            )
                            nc.register_instruction(nop)
                            new_list.append(nop)
                        si.on_wait = [waits[-1]]
                        changed = True
                new_list.append(inst)
            if changed:
                bb.instructions = new_list
    return counter


# ---------------------------------------------------------------------------
# Kernel build
# ---------------------------------------------------------------------------
def _build_nc():
    import concourse.bass as bass
    import concourse.mybir as mybir
    from concourse.tile import TileContext

    f32 = mybir.dt.float32
    f16 = mybir.dt.float16

    nc = bass.Bass()
    wt_d = nc.declare_dram_parameter("wt", [NVH, NK, P, VH], f16, isOutput=False)
    xtt_d = nc.declare_dram_parameter("xtt", [NRB, P, NK * P], f16, isOutput=False)
    brep_d = nc.declare_dram_parameter("brep", [P, V], f16, isOutput=False)
    vrep_d = nc.declare_dram_parameter("vrep", [P, V], f16, isOutput=False)
    # raw pre-softmax scores, u_d[q, rb] = u[row 128*rb+q]
    u_d = nc.declare_dram_parameter("u", [P, NRB], f32, isOutput=True)

    with TileContext(nc) as tc:
        with (
            tc.tile_pool(name="wpool", bufs=1) as wpool,
            tc.tile_pool(name="xpool", bufs=1) as xpool,
            tc.tile_pool(name="tpool", bufs=1) as tpool,
            tc.tile_pool(name="spool", bufs=1) as spool,
            tc.tile_pool(name="pspool", bufs=1, space="PSUM") as pspool,
        ):
            # --- constants (scalar queue: fp16 so they cost little of the
            # critical early HBM bandwidth) ---
            brep_sb = spool.tile([P, V], f16, name="brep_sb")
            nc.scalar.dma_start(out=brep_sb[:], in_=brep_d[:, :])
            vrep_sb = spool.tile([P, V], f16, name="vrep_sb")
            nc.scalar.dma_start(out=vrep_sb[:], in_=vrep_d[:, :])

            u_all = spool.tile([P, NRB], f32, name="u_all")
            # per-rowblock v-half partials (rotating set of 4)
            pu_all = [
                spool.tile([P, NVH], f32, name=f"pu{i}") for i in range(4)
            ]

            # --- W in [vh][k-chunk] tiles; vh0's first chunks are small so
            # the very first matmuls gate on less data ---
            W_CHUNKS = {0: [4, 4, 8, 8, 8], 1: [8, 8, 8, 8]}
            wt_tiles = {0: [], 1: []}

            def load_w_chunk(vh, ci):
                k0 = sum(W_CHUNKS[vh][:ci])
                kw = W_CHUNKS[vh][ci]
                t = wpool.tile([P, kw, VH], f16, name=f"w_{vh}_{ci}")
                nc.sync.dma_start(
                    out=t[:],
                    in_=wt_d[vh, k0 : k0 + kw].rearrange("k p n -> p k n"),
                )
                wt_tiles[vh].append((k0, kw, t))

            def wt_tile(vh, k):
                for k0, kw, t in wt_tiles[vh]:
                    if k0 <= k < k0 + kw:
                        return t[:, k - k0]
                raise AssertionError(k)

            # --- xtt row-block tiles (ring of 6) ---
            xtt_tiles = {}

            def load_xtt(rb, engine):
                t = xpool.tile(
                    [P, NK, P], f16, name=f"xtt_{rb}", tag="xt", bufs=6
                )
                getattr(nc, engine).dma_start(
                    out=t[:],
                    in_=xtt_d[rb].rearrange("p (k c) -> p k c", c=P),
                )
                xtt_tiles[rb] = t

            # head triggers on sync in priority order: the first chains
            # (rb0..3, vh0) consume k-chunk-major, so feed xtt0..3 + w(0,*)
            # first, then w(1,*), then the next xtt blocks
            for step in [
                ("x", 0), ("w", 0, 0), ("w", 0, 1), ("x", 1), ("w", 0, 2),
                ("x", 2), ("w", 0, 3), ("x", 3), ("w", 0, 4), ("w", 1, 0),
                ("w", 1, 1), ("w", 1, 2), ("w", 1, 3), ("x", 4), ("x", 5),
            ]:
                if step[0] == "w":
                    load_w_chunk(step[1], step[2])
                else:
                    load_xtt(step[1], "sync")

            def new_ps():
                return pspool.tile([P, VH], f32, name="ps", tag="ps", bufs=6)

            def consume(rb, vh, ps):
                """psum [rows, v] -> bias add (DVE), tanh (ACT), weighted
                free-axis reduce into u_all[:, rb] (DVE)."""
                sl = slice(vh * VH, (vh + 1) * VH)
                tb = tpool.tile([P, VH], f32, name="tb", tag="tb", bufs=4)
                nc.vector.tensor_add(tb[:], ps[:], brep_sb[:, sl])
                nc.scalar.activation(
                    tb[:], tb[:], mybir.ActivationFunctionType.Tanh
                )
                nc.vector.tensor_mul(tb[:], tb[:], vrep_sb[:, sl])
                pu = pu_all[rb % 4]
                nc.vector.tensor_reduce(
                    pu[0:P, vh : vh + 1],
                    tb[:],
                    mybir.AxisListType.X,
                    mybir.AluOpType.add,
                )
                if vh == 1:
                    nc.vector.tensor_add(
                        u_all[:, rb : rb + 1], pu[0:P, 0:1], pu[0:P, 1:2]
                    )
                if vh == 1 and (rb + 1) % RPB == 0:
                    # batch rb // RPB complete: ship its raw scores
                    b = rb // RPB
                    nc.scalar.dma_start(
                        out=u_d[:, b * RPB : (b + 1) * RPB],
                        in_=u_all[:, b * RPB : (b + 1) * RPB],
                    )

            def emit_chain(rb, vh, ps):
                for k in range(NK):
                    nc.tensor.matmul(
                        ps[:],
                        xtt_tiles[rb][:, k],
                        wt_tile(vh, k),
                        start=(k == 0),
                        stop=(k == NK - 1),
                    )

            # head phases: rb0..3 k-chunk-major per v-half (4 psum banks in
            # flight -> 4x work per arrived W chunk)
            for vh in range(NVH):
                pss = [new_ps() for _ in range(4)]
                for k0, kw, _t in wt_tiles[vh]:
                    for rb in range(4):
                        for k in range(k0, k0 + kw):
                            nc.tensor.matmul(
                                pss[rb][:],
                                xtt_tiles[rb][:, k],
                                wt_tile(vh, k),
                                start=(k == 0),
                                stop=(k == NK - 1),
                            )
                for rb in range(4):
                    consume(rb, vh, pss[rb])
            for rb in (6, 7):
                load_xtt(rb, "gpsimd")

            # steady state
            for rb in range(4, NRB):
                for vh in range(NVH):
                    ps = new_ps()
                    emit_chain(rb, vh, ps)
                    consume(rb, vh, ps)
                if rb + 4 < NRB:
                    load_xtt(rb + 4, "gpsimd")
                del xtt_tiles[rb]

    _split_multiwait(nc)
    return nc


# ---------------------------------------------------------------------------
# Host entry point
# ---------------------------------------------------------------------------
def kernel(hidden, z, Ww, bw, Wz, bz, Vw, vb, w_a):
    _install_drain_patch()
    from concourse.bass_utils import run_bass_kernel_spmd

    np_dt = np.float16

    # ---- host-side shard prep ----
    hid_t = np.ascontiguousarray(
        np.asarray(hidden).astype(np_dt).transpose(2, 1, 0)
    )  # [H2, B, S]
    z_t = np.ascontiguousarray(
        np.asarray(z).astype(np_dt).transpose(2, 1, 0)
    )  # [H2, B, S]

    w_cat = np.concatenate(
        [np.asarray(Ww), np.asarray(Wz)], axis=0
    ).astype(np_dt)  # [H, V]
    # wt[vh, k, p, n] = W[128k+p, 512vh+n]
    wt = np.ascontiguousarray(
        w_cat.reshape(NK, P, NVH, VH).transpose(2, 0, 1, 3)
    )  # [NVH, NK, P, VH]

    bias = (
        np.asarray(bw).astype(np.float64)
        + np.asarray(bz).astype(np.float64)
        + float(np.asarray(w_a)) * ALPHA_S
    ).astype(np.float32)  # [V]
    brep = np.ascontiguousarray(np.broadcast_to(bias.astype(np_dt), (P, V)))
    vrep = np.ascontiguousarray(
        np.broadcast_to(np.asarray(Vw).astype(np_dt).reshape(V), (P, V))
    )

    in_maps = []
    for c in range(NCORES):
        xt_c = np.empty((H, ROWS), dtype=np_dt)  # [H, ROWS]
        xt_c[:H2] = hid_t[:, 2 * c : 2 * c + 2, :].reshape(H2, ROWS)
        xt_c[H2:] = z_t[:, 2 * c : 2 * c + 2, :].reshape(H2, ROWS)
        # xtt[rb, p, 128k+cc] = X[128 rb + cc, 128 k + p] = xt_c[128k+p, 128rb+cc]
        xtt = np.ascontiguousarray(
            xt_c.reshape(NK, P, NRB, P).transpose(2, 1, 0, 3)
        ).reshape(NRB, P, NK * P)
        in_maps.append({"xtt": xtt, "wt": wt, "brep": brep, "vrep": vrep})

    nc = _build_nc()
    res = run_bass_kernel_spmd(nc, in_maps, list(range(NCORES)))

    # gather raw scores, then softmax over s per batch column (host-side
    # epilogue on [S, B] -- 32K values). u_d[q, rb] = u[row 128*rb+q],
    # row = b*2048 + s.
    u = np.empty((S, B), dtype=np.float64)
    for c in range(NCORES):
        uc = np.asarray(res.results[c]["u"], dtype=np.float64)  # [P, NRB]
        loc = uc.T.reshape(BC, S)  # [b, s]
        for b in range(BC):
            u[:, 2 * c + b] = loc[b]
    u -= u.max(axis=0, keepdims=True)
    e = np.exp(u)
    out = (e / e.sum(axis=0, keepdims=True)).astype(np.float32)
    return out[:, :, None]
